# revision 21
# baseline (speedup 1.0000x reference)
"""Trainium2 Bass kernel: EuclideanCodebook (VQ) forward.

Contract: kernel(x, embed) takes the FULL inputs
    x [8, 4096, 512] f32, embed [2048, 512] f32
and returns (quantize [8, 4096, 512] f32, embed_ind [8, 4096] i32),
matching the eval-mode EuclideanCodebook reference:
    ind = argmax_c -(||x||^2 - 2 x.e_c + ||e_c||^2);  quantize = embed[ind]

Sharding: data-parallel over the batch axis — core i handles x[i]
(4096 tokens); the codebook is replicated on every core.

Per-core algorithm:
  * scores[t, c] = x_t . e_c - ||e_c||^2 / 2   (argmax-equivalent)
  * The matmul runs on the PE as a 3-pass bf16 hi/lo decomposition
    (x_hi.e_hi + x_lo.e_hi + x_hi.e_lo), accumulated in fp32 PSUM.
    On the fixed inputs this keeps the true argmax ahead by >= 2.4e-4
    per token, while single-pass reduced precision would flip many
    near-ties.
  * x tiles are cast on ACT, the lo residual computed on GpSimd, and
    hi|lo transposed to [d, tok] in one XBAR transpose DMA.
  * argmax over the 2048 scores per token uses DVE max8/find_index8
    (ties resolve to the lowest index, like jnp.argmax).
  * quantize rows are fetched with an indirect gather DMA from the
    original fp32 codebook, so output rows are bit-exact.

The emission order software-pipelines tile preparation PIPE tiles ahead
so every engine FIFO (ACT casts, GpSimd lo-sub, sync transposes) stays
ahead of the PE; the PE then streams matmuls back-to-back at the
~216 ns/MM roofline without HAM re-throttle stalls.
"""

import numpy as np
import ml_dtypes

B, T, D, C = 8, 4096, 512, 2048
TPT = 128            # tokens per tile (partition dim)
NT = T // TPT        # token tiles per core
KT = D // 128        # contraction k-tiles
CCH = 512            # codes per PSUM chunk
NCH = C // CCH       # code chunks
PIPE = 4             # prep-ahead depth (tiles)
LAG_G = 2            # gather lags the argmax by this many tiles
LAG_S = 3            # output stores lag by this many tiles

_CACHE = {}
LAST_RESULTS = None


def _build():
    import concourse.mybir as mybir
    import concourse.tile as tile
    import concourse.bass as bass
    from concourse import bacc

    dt = mybir.dt
    nc = bacc.Bacc("TRN2", target_bir_lowering=False, debug=False, num_devices=8)

    x_d = nc.dram_tensor("x", [T, D], dt.float32, kind="ExternalInput")
    eh_d = nc.dram_tensor("ehT", [D, C], dt.bfloat16, kind="ExternalInput")
    el_d = nc.dram_tensor("elT", [D, C], dt.bfloat16, kind="ExternalInput")
    esq_d = nc.dram_tensor("esq3", [3, C], dt.bfloat16, kind="ExternalInput")
    emb_d = nc.dram_tensor("embed", [C, D], dt.float32, kind="ExternalInput")
    q_d = nc.dram_tensor("q", [T, D], dt.float32, kind="ExternalOutput")
    i_d = nc.dram_tensor("ind", [T, 1], dt.uint32, kind="ExternalOutput")

    x_ap = x_d.ap().rearrange("(i p) d -> i p d", p=TPT)
    q_ap = q_d.ap().rearrange("(i p) d -> i p d", p=TPT)
    ind_ap = i_d.ap().rearrange("(i p) o -> i p o", p=TPT)
    eh_view = eh_d.ap().rearrange("(k p) c -> k p c", p=128)
    el_view = el_d.ap().rearrange("(k p) c -> k p c", p=128)

    with tile.TileContext(nc) as tc:
        with (
            tc.tile_pool(name="const", bufs=1) as cpool,
            tc.tile_pool(name="prep", bufs=PIPE + 1) as wpool,
            tc.tile_pool(name="sc", bufs=2) as scpool,
            tc.tile_pool(name="qo", bufs=LAG_S + 3) as qpool,
            tc.tile_pool(name="ps", bufs=2, space="PSUM") as ppool,
        ):
            eh_sb = cpool.tile([128, KT, C], dt.bfloat16)
            el_sb = cpool.tile([128, KT, C], dt.bfloat16)
            # -||e||^2/2 as 3 bf16 terms, applied on the PE via a K=3
            # matmul against a ones stationary so the DVE never touches it
            esq_sb = cpool.tile([3, C], dt.bfloat16)
            ones_sb = cpool.tile([3, TPT], dt.bfloat16)
            nc.vector.memset(ones_sb[:], 1.0)

            xTs = {}

            def prep(j):
                # load -> split hi/lo -> transpose; touches only
                # ACT / GpSimd-front / sync, never the DVE
                xt = wpool.tile([TPT, D], dt.float32, name=f"xt{j}", tag="xt")
                nc.scalar.dma_start(xt[:], x_ap[j])
                hl = wpool.tile([TPT, 2, D], dt.bfloat16, name=f"hl{j}", tag="hl")
                nc.scalar.copy(hl[:, 0, :], xt[:])
                hi32 = wpool.tile([TPT, D], dt.float32, name=f"h32{j}", tag="h32")
                nc.scalar.copy(hi32[:], hl[:, 0, :])
                nc.gpsimd.tensor_sub(hl[:, 1, :], xt[:], hi32[:])
                xT = wpool.tile(
                    [128, 2 * KT, TPT], dt.bfloat16,
                    name=f"xT{j}", tag="xT", bufs=PIPE + 2,
                )
                nc.sync.dma_start_transpose(xT[:], hl[:])
                return xT

            # startup: tile 0's chain + the first codebook slice go first
            nc.sync.dma_start(eh_sb[:, 0, :], eh_view[0])
            xTs[0] = prep(0)
            for k in range(1, KT):
                nc.sync.dma_start(eh_sb[:, k, :], eh_view[k])
            for k in range(KT):
                nc.scalar.dma_start(el_sb[:, k, :], el_view[k])
            nc.gpsimd.dma_start(esq_sb[:], esq_d.ap())
            for j in range(1, min(PIPE, NT)):
                xTs[j] = prep(j)

            # accumulation steps (x-side m-tile, e-side sbuf, k):
            # hi.e_hi, lo.e_hi, hi.e_lo for each k
            STEPS = []
            for k in range(KT):
                STEPS += [(k, eh_sb, k), (KT + k, eh_sb, k), (k, el_sb, k)]

            idx8s = {}

            def gather(j):
                qt = qpool.tile([TPT, D], dt.float32, name=f"qt{j}", tag="qt")
                nc.gpsimd.indirect_dma_start(
                    out=qt[:],
                    out_offset=None,
                    in_=emb_d.ap(),
                    in_offset=bass.IndirectOffsetOnAxis(ap=idx8s[j][:, :1], axis=0),
                )
                return qt

            qts = {}

            def store(j):
                nc.scalar.dma_start(q_ap[j], qts.pop(j)[:])
                nc.sync.dma_start(ind_ap[j], idx8s.pop(j)[:, :1])

            for i in range(NT):
                if i + PIPE < NT:
                    xTs[i + PIPE] = prep(i + PIPE)

                xT = xTs.pop(i)
                ps = ppool.tile([TPT, C], dt.float32)
                # step 0: ps = -||e||^2/2 via ones.T @ esq3 (K=3)
                for c in range(NCH):
                    nc.tensor.matmul(
                        ps[:, c * CCH:(c + 1) * CCH],
                        ones_sb[:],
                        esq_sb[:, c * CCH:(c + 1) * CCH],
                        start=True,
                        stop=False,
                    )
                for s, (m, esb, k) in enumerate(STEPS):
                    for c in range(NCH):
                        nc.tensor.matmul(
                            ps[:, c * CCH:(c + 1) * CCH],
                            xT[:, m, :],
                            esb[:, k, c * CCH:(c + 1) * CCH],
                            start=False,
                            stop=(s == len(STEPS) - 1),
                        )

                m8 = scpool.tile([TPT, 8], dt.float32, bufs=LAG_S + 3)
                idx8 = scpool.tile([TPT, 8], dt.uint32, bufs=LAG_S + 3)
                nc.vector.max(out=m8[:], in_=ps[:])
                nc.vector.max_index(idx8[:], m8[:], ps[:])
                idx8s[i] = idx8

                # consumers of the argmax run LAGGED so they never block
                # an engine FIFO on the current tile's DVE tail
                if i >= LAG_G:
                    qts[i - LAG_G] = gather(i - LAG_G)
                if i >= LAG_S:
                    store(i - LAG_S)

            for j in range(NT - LAG_G, NT):
                qts[j] = gather(j)
            for j in range(NT - LAG_S, NT):
                store(j)

    nc.compile()
    return nc


def _host_consts(embed):
    bf = ml_dtypes.bfloat16
    e = np.ascontiguousarray(np.asarray(embed, dtype=np.float32))
    e_hi = e.astype(bf)
    e_lo = (e - e_hi.astype(np.float32)).astype(bf)
    ehT = np.ascontiguousarray(e_hi.T)
    elT = np.ascontiguousarray(e_lo.T)
    # -||e||^2/2 as three cascading bf16 terms (exact to ~2^-27 rel)
    t = (-0.5 * (e.astype(np.float64) ** 2).sum(1)).astype(np.float32)
    esq3 = np.zeros((3, C), dtype=bf)
    r = t
    for j in range(3):
        esq3[j] = r.astype(bf)
        r = r - esq3[j].astype(np.float32)
    return e, ehT, elT, esq3


def _in_maps(x, embed):
    e, ehT, elT, esq3 = _host_consts(embed)
    xs = np.asarray(x, dtype=np.float32).reshape(B, T, D)
    return [
        {
            "x": np.ascontiguousarray(xs[i]),
            "ehT": ehT,
            "elT": elT,
            "esq3": esq3,
            "embed": e,
        }
        for i in range(B)
    ]


def kernel(x, embed):
    global LAST_RESULTS
    from concourse import bass_utils

    if "nc" not in _CACHE:
        _CACHE["nc"] = _build()
    nc = _CACHE["nc"]

    res = bass_utils.run_bass_kernel_spmd(
        nc, _in_maps(x, embed), core_ids=list(range(B))
    )
    LAST_RESULTS = res
    outs = res.results
    q = np.stack([np.asarray(outs[i]["q"]).reshape(T, D) for i in range(B)], 0)
    ind = np.stack(
        [np.asarray(outs[i]["ind"]).reshape(T).astype(np.int32) for i in range(B)], 0
    )
    return q.astype(np.float32, copy=False), ind


# revision 23
# speedup vs baseline: 1.2755x; 1.2755x over previous
"""Trainium2 Bass kernel: EuclideanCodebook (VQ) forward.

Contract: kernel(x, embed) takes the FULL inputs
    x [8, 4096, 512] f32, embed [2048, 512] f32
and returns (quantize [8, 4096, 512] f32, embed_ind [8, 4096] i32),
matching the eval-mode EuclideanCodebook reference:
    ind = argmax_c -(||x||^2 - 2 x.e_c + ||e_c||^2);  quantize = embed[ind]

Sharding: data-parallel over the batch axis — core i handles x[i]
(4096 tokens); the codebook is replicated on every core.

Per-core algorithm:
  * scores[t, c] = x_t . e_c - ||e_c||^2 / 2   (argmax-equivalent)
  * The matmul runs on the PE as a 3-pass bf16 hi/lo decomposition
    (x_hi.e_hi + x_lo.e_hi + x_hi.e_lo), accumulated in fp32 PSUM.
    On the fixed inputs this keeps the true argmax ahead by >= 2.4e-4
    per token, while single-pass reduced precision would flip many
    near-ties.
  * x tiles are cast on ACT, the lo residual computed on GpSimd, and
    hi|lo transposed to [d, tok] in one XBAR transpose DMA.
  * argmax over the 2048 scores per token uses DVE max8/find_index8
    (ties resolve to the lowest index, like jnp.argmax).
  * quantize rows are fetched with an indirect gather DMA from the
    original fp32 codebook, so output rows are bit-exact.

The emission order software-pipelines tile preparation PIPE tiles ahead
so every engine FIFO (ACT casts, GpSimd lo-sub, sync transposes) stays
ahead of the PE; the PE then streams matmuls back-to-back at the
~216 ns/MM roofline without HAM re-throttle stalls.
"""

import numpy as np
import ml_dtypes

B, T, D, C = 8, 4096, 512, 2048
TPT = 128            # tokens per tile (partition dim)
NT = T // TPT        # token tiles per core
KT = D // 128        # contraction k-tiles
CCH = 512            # codes per PSUM chunk
NCH = C // CCH       # code chunks
PIPE = 4             # prep-ahead depth (tiles)
LAG_G = 2            # gather lags the argmax by this many tiles
LAG_S = 3            # output stores lag by this many tiles

_CACHE = {}
LAST_RESULTS = None


def _build():
    import concourse.mybir as mybir
    import concourse.tile as tile
    import concourse.bass as bass
    from concourse import bacc

    dt = mybir.dt
    nc = bacc.Bacc("TRN2", target_bir_lowering=False, debug=False, num_devices=8)

    x_d = nc.dram_tensor("x", [T, D], dt.float32, kind="ExternalInput")
    eh_d = nc.dram_tensor("ehT", [D, C], dt.bfloat16, kind="ExternalInput")
    el_d = nc.dram_tensor("elT", [D, C], dt.bfloat16, kind="ExternalInput")
    esq_d = nc.dram_tensor("esq3", [3, C], dt.bfloat16, kind="ExternalInput")
    emb_d = nc.dram_tensor("embed", [C, D], dt.float32, kind="ExternalInput")
    q_d = nc.dram_tensor("q", [T, D], dt.float32, kind="ExternalOutput")
    i_d = nc.dram_tensor("ind", [T, 1], dt.uint32, kind="ExternalOutput")

    x_ap = x_d.ap().rearrange("(i p) d -> i p d", p=TPT)
    q_ap = q_d.ap().rearrange("(i p) d -> i p d", p=TPT)
    ind_ap = i_d.ap().rearrange("(i p) o -> i p o", p=TPT)
    eh_view = eh_d.ap().rearrange("(k p) c -> k p c", p=128)
    el_view = el_d.ap().rearrange("(k p) c -> k p c", p=128)

    with tile.TileContext(nc) as tc:
        with (
            tc.tile_pool(name="const", bufs=1) as cpool,
            tc.tile_pool(name="prep", bufs=PIPE + 1) as wpool,
            tc.tile_pool(name="sc", bufs=2) as scpool,
            tc.tile_pool(name="qo", bufs=12) as qpool,
            tc.tile_pool(name="ps", bufs=2, space="PSUM") as ppool,
        ):
            eh_sb = cpool.tile([128, KT, C], dt.bfloat16)
            el_sb = cpool.tile([128, KT, C], dt.bfloat16)
            # -||e||^2/2 as 3 bf16 terms, applied on the PE via a K=3
            # matmul against a ones stationary so the DVE never touches it
            esq_sb = cpool.tile([3, C], dt.bfloat16)
            ones_sb = cpool.tile([3, TPT], dt.bfloat16)
            nc.vector.memset(ones_sb[:], 1.0)

            xTs = {}

            def prep(j):
                # load -> split hi/lo -> transpose; touches only
                # ACT / GpSimd-front / sync, never the DVE
                xt = wpool.tile([TPT, D], dt.float32, name=f"xt{j}", tag="xt")
                nc.scalar.dma_start(xt[:], x_ap[j])
                hl = wpool.tile([TPT, 2, D], dt.bfloat16, name=f"hl{j}", tag="hl")
                nc.scalar.copy(hl[:, 0, :], xt[:])
                hi32 = wpool.tile([TPT, D], dt.float32, name=f"h32{j}", tag="h32")
                nc.scalar.copy(hi32[:], hl[:, 0, :])
                nc.gpsimd.tensor_sub(hl[:, 1, :], xt[:], hi32[:])
                xT = wpool.tile(
                    [128, 2 * KT, TPT], dt.bfloat16,
                    name=f"xT{j}", tag="xT", bufs=NT,
                )
                nc.sync.dma_start_transpose(xT[:], hl[:])
                return xT

            # ALL tile prep is emitted up front: the whole transposed
            # shard lives in SBUF (64KB/partition), so no consumer ever
            # precedes a prep op in any engine FIFO
            nc.sync.dma_start(eh_sb[:, 0, :], eh_view[0])
            xTs[0] = prep(0)
            for k in range(1, KT):
                nc.sync.dma_start(eh_sb[:, k, :], eh_view[k])
            for k in range(KT):
                nc.scalar.dma_start(el_sb[:, k, :], el_view[k])
            nc.gpsimd.dma_start(esq_sb[:], esq_d.ap())
            for j in range(1, NT):
                xTs[j] = prep(j)

            # accumulation steps (x-side m-tile, e-side sbuf, k):
            # hi.e_hi, lo.e_hi, hi.e_lo for each k
            STEPS = []
            for k in range(KT):
                STEPS += [(k, eh_sb, k), (KT + k, eh_sb, k), (k, el_sb, k)]

            idx8s = {}

            def gather(j):
                qt = qpool.tile([TPT, D], dt.float32, name=f"qt{j}", tag="qt")
                nc.gpsimd.indirect_dma_start(
                    out=qt[:],
                    out_offset=None,
                    in_=emb_d.ap(),
                    in_offset=bass.IndirectOffsetOnAxis(ap=idx8s[j][:, :1], axis=0),
                )
                return qt

            qts = {}

            def store(j):
                nc.scalar.dma_start(q_ap[j], qts.pop(j)[:])
                nc.sync.dma_start(ind_ap[j], idx8s.pop(j)[:, :1])

            for i in range(NT):
                xT = xTs.pop(i)
                ps = ppool.tile([TPT, C], dt.float32)
                # step 0: ps = -||e||^2/2 via ones.T @ esq3 (K=3)
                for c in range(NCH):
                    nc.tensor.matmul(
                        ps[:, c * CCH:(c + 1) * CCH],
                        ones_sb[:],
                        esq_sb[:, c * CCH:(c + 1) * CCH],
                        start=True,
                        stop=False,
                    )
                for s, (m, esb, k) in enumerate(STEPS):
                    for c in range(NCH):
                        nc.tensor.matmul(
                            ps[:, c * CCH:(c + 1) * CCH],
                            xT[:, m, :],
                            esb[:, k, c * CCH:(c + 1) * CCH],
                            start=False,
                            stop=(s == len(STEPS) - 1),
                        )

                m8 = scpool.tile([TPT, 8], dt.float32, bufs=NT)
                idx8 = scpool.tile([TPT, 8], dt.uint32, bufs=NT)
                nc.vector.max(out=m8[:], in_=ps[:])
                nc.vector.max_index(idx8[:], m8[:], ps[:])
                idx8s[i] = idx8

                # consumers of the argmax run LAGGED so they never block
                # an engine FIFO on the current tile's DVE tail
                if i >= LAG_G:
                    qts[i - LAG_G] = gather(i - LAG_G)
                if i >= LAG_S:
                    store(i - LAG_S)

            for j in range(NT - LAG_G, NT):
                qts[j] = gather(j)
            for j in range(NT - LAG_S, NT):
                store(j)

    nc.compile()
    return nc


def _host_consts(embed):
    bf = ml_dtypes.bfloat16
    e = np.ascontiguousarray(np.asarray(embed, dtype=np.float32))
    e_hi = e.astype(bf)
    e_lo = (e - e_hi.astype(np.float32)).astype(bf)
    ehT = np.ascontiguousarray(e_hi.T)
    elT = np.ascontiguousarray(e_lo.T)
    # -||e||^2/2 as three cascading bf16 terms (exact to ~2^-27 rel)
    t = (-0.5 * (e.astype(np.float64) ** 2).sum(1)).astype(np.float32)
    esq3 = np.zeros((3, C), dtype=bf)
    r = t
    for j in range(3):
        esq3[j] = r.astype(bf)
        r = r - esq3[j].astype(np.float32)
    return e, ehT, elT, esq3


def _in_maps(x, embed):
    e, ehT, elT, esq3 = _host_consts(embed)
    xs = np.asarray(x, dtype=np.float32).reshape(B, T, D)
    return [
        {
            "x": np.ascontiguousarray(xs[i]),
            "ehT": ehT,
            "elT": elT,
            "esq3": esq3,
            "embed": e,
        }
        for i in range(B)
    ]


def kernel(x, embed):
    global LAST_RESULTS
    from concourse import bass_utils

    if "nc" not in _CACHE:
        _CACHE["nc"] = _build()
    nc = _CACHE["nc"]

    res = bass_utils.run_bass_kernel_spmd(
        nc, _in_maps(x, embed), core_ids=list(range(B))
    )
    LAST_RESULTS = res
    outs = res.results
    q = np.stack([np.asarray(outs[i]["q"]).reshape(T, D) for i in range(B)], 0)
    ind = np.stack(
        [np.asarray(outs[i]["ind"]).reshape(T).astype(np.int32) for i in range(B)], 0
    )
    return q.astype(np.float32, copy=False), ind


# revision 24
# speedup vs baseline: 1.2810x; 1.0043x over previous
"""Trainium2 Bass kernel: EuclideanCodebook (VQ) forward.

Contract: kernel(x, embed) takes the FULL inputs
    x [8, 4096, 512] f32, embed [2048, 512] f32
and returns (quantize [8, 4096, 512] f32, embed_ind [8, 4096] i32),
matching the eval-mode EuclideanCodebook reference:
    ind = argmax_c -(||x||^2 - 2 x.e_c + ||e_c||^2);  quantize = embed[ind]

Sharding: data-parallel over the batch axis — core i handles x[i]
(4096 tokens); the codebook is replicated on every core.

Per-core algorithm:
  * scores[t, c] = x_t . e_c - ||e_c||^2 / 2   (argmax-equivalent)
  * The matmul runs on the PE as a 3-pass bf16 hi/lo decomposition
    (x_hi.e_hi + x_lo.e_hi + x_hi.e_lo), accumulated in fp32 PSUM.
    On the fixed inputs this keeps the true argmax ahead by >= 2.4e-4
    per token, while single-pass reduced precision would flip many
    near-ties.
  * x tiles are cast on ACT, the lo residual computed on GpSimd, and
    hi|lo transposed to [d, tok] in one XBAR transpose DMA.
  * argmax over the 2048 scores per token uses DVE max8/find_index8
    (ties resolve to the lowest index, like jnp.argmax).
  * quantize rows are fetched with an indirect gather DMA from the
    original fp32 codebook, so output rows are bit-exact.

All 32 tiles' preparation (load/split/transpose) is emitted before the
matmul loop and the whole transposed shard stays in SBUF, so no prep op
ever queues behind an argmax/gather consumer in any engine FIFO; the PE
then streams matmuls back-to-back at the ~216 ns/MM issue roofline
without HAM re-throttle stalls.  The argmax consumers (indirect gather,
output stores) run a few tiles LAGGED so their semaphore waits are
pre-satisfied when they reach an engine queue head.
"""

import numpy as np
import ml_dtypes

B, T, D, C = 8, 4096, 512, 2048
TPT = 128            # tokens per tile (partition dim)
NT = T // TPT        # token tiles per core
KT = D // 128        # contraction k-tiles
CCH = 512            # codes per PSUM chunk
NCH = C // CCH       # code chunks
PIPE = 4             # prep-ahead depth (tiles)
LAG_G = 2            # gather lags the argmax by this many tiles
LAG_S = 3            # output stores lag by this many tiles

_CACHE = {}
LAST_RESULTS = None


def _build():
    import concourse.mybir as mybir
    import concourse.tile as tile
    import concourse.bass as bass
    from concourse import bacc

    dt = mybir.dt
    nc = bacc.Bacc("TRN2", target_bir_lowering=False, debug=False, num_devices=8)

    x_d = nc.dram_tensor("x", [T, D], dt.float32, kind="ExternalInput")
    eh_d = nc.dram_tensor("ehT", [D, C], dt.bfloat16, kind="ExternalInput")
    el_d = nc.dram_tensor("elT", [D, C], dt.bfloat16, kind="ExternalInput")
    esq_d = nc.dram_tensor("esq3", [3, C], dt.bfloat16, kind="ExternalInput")
    emb_d = nc.dram_tensor("embed", [C, D], dt.float32, kind="ExternalInput")
    q_d = nc.dram_tensor("q", [T, D], dt.float32, kind="ExternalOutput")
    i_d = nc.dram_tensor("ind", [T, 1], dt.uint32, kind="ExternalOutput")

    x_ap = x_d.ap().rearrange("(i p) d -> i p d", p=TPT)
    q_ap = q_d.ap().rearrange("(i p) d -> i p d", p=TPT)
    ind_ap = i_d.ap().rearrange("(i p) o -> i p o", p=TPT)
    eh_view = eh_d.ap().rearrange("(k p) c -> k p c", p=128)
    el_view = el_d.ap().rearrange("(k p) c -> k p c", p=128)

    with tile.TileContext(nc) as tc:
        with (
            tc.tile_pool(name="const", bufs=1) as cpool,
            tc.tile_pool(name="prep", bufs=PIPE + 1) as wpool,
            tc.tile_pool(name="sc", bufs=2) as scpool,
            tc.tile_pool(name="qo", bufs=12) as qpool,
            tc.tile_pool(name="ps", bufs=2, space="PSUM") as ppool,
        ):
            eh_sb = cpool.tile([128, KT, C], dt.bfloat16)
            el_sb = cpool.tile([128, KT, C], dt.bfloat16)
            # -||e||^2/2 as 3 bf16 terms, applied on the PE via a K=3
            # matmul against a ones stationary so the DVE never touches it
            esq_sb = cpool.tile([3, C], dt.bfloat16)
            ones_sb = cpool.tile([3, TPT], dt.bfloat16)
            nc.vector.memset(ones_sb[:], 1.0)

            xTs = {}

            def prep(j):
                # load -> split hi/lo -> transpose; touches only
                # ACT / GpSimd-front / sync, never the DVE
                xt = wpool.tile([TPT, D], dt.float32, name=f"xt{j}", tag="xt")
                nc.scalar.dma_start(xt[:], x_ap[j])
                hl = wpool.tile([TPT, 2, D], dt.bfloat16, name=f"hl{j}", tag="hl")
                nc.scalar.copy(hl[:, 0, :], xt[:])
                hi32 = wpool.tile([TPT, D], dt.float32, name=f"h32{j}", tag="h32")
                nc.scalar.copy(hi32[:], hl[:, 0, :])
                nc.gpsimd.tensor_sub(hl[:, 1, :], xt[:], hi32[:])
                xT = wpool.tile(
                    [128, 2 * KT, TPT], dt.bfloat16,
                    name=f"xT{j}", tag="xT", bufs=NT,
                )
                nc.sync.dma_start_transpose(xT[:], hl[:])
                return xT

            # ALL tile prep is emitted up front: the whole transposed
            # shard lives in SBUF (64KB/partition), so no consumer ever
            # precedes a prep op in any engine FIFO
            nc.sync.dma_start(eh_sb[:, 0, :], eh_view[0])
            xTs[0] = prep(0)
            for k in range(1, KT):
                nc.sync.dma_start(eh_sb[:, k, :], eh_view[k])
            for k in range(KT):
                nc.scalar.dma_start(el_sb[:, k, :], el_view[k])
            nc.gpsimd.dma_start(esq_sb[:], esq_d.ap())
            for j in range(1, NT):
                xTs[j] = prep(j)

            # accumulation steps (x-side m-tile, e-side sbuf, k):
            # hi.e_hi, lo.e_hi, hi.e_lo for each k
            STEPS = []
            for k in range(KT):
                STEPS += [(k, eh_sb, k), (KT + k, eh_sb, k), (k, el_sb, k)]

            idx8s = {}

            def gather(j):
                qt = qpool.tile([TPT, D], dt.float32, name=f"qt{j}", tag="qt")
                nc.gpsimd.indirect_dma_start(
                    out=qt[:],
                    out_offset=None,
                    in_=emb_d.ap(),
                    in_offset=bass.IndirectOffsetOnAxis(ap=idx8s[j][:, :1], axis=0),
                )
                return qt

            qts = {}

            def store(j):
                nc.scalar.dma_start(q_ap[j], qts.pop(j)[:])
                nc.sync.dma_start(ind_ap[j], idx8s.pop(j)[:, :1])

            for i in range(NT):
                xT = xTs.pop(i)
                ps = ppool.tile([TPT, C], dt.float32)
                # step 0: ps = -||e||^2/2 via ones.T @ esq3 (K=3)
                for c in range(NCH):
                    nc.tensor.matmul(
                        ps[:, c * CCH:(c + 1) * CCH],
                        ones_sb[:],
                        esq_sb[:, c * CCH:(c + 1) * CCH],
                        start=True,
                        stop=False,
                    )
                for s, (m, esb, k) in enumerate(STEPS):
                    for c in range(NCH):
                        nc.tensor.matmul(
                            ps[:, c * CCH:(c + 1) * CCH],
                            xT[:, m, :],
                            esb[:, k, c * CCH:(c + 1) * CCH],
                            start=False,
                            stop=(s == len(STEPS) - 1),
                        )

                m8 = scpool.tile([TPT, 8], dt.float32, bufs=NT)
                idx8 = scpool.tile([TPT, 8], dt.uint32, bufs=NT)
                nc.vector.max(out=m8[:], in_=ps[:])
                nc.vector.max_index(idx8[:], m8[:], ps[:])
                idx8s[i] = idx8

                # consumers of the argmax run LAGGED so they never block
                # an engine FIFO on the current tile's DVE tail
                if i >= LAG_G:
                    qts[i - LAG_G] = gather(i - LAG_G)
                if i >= LAG_S:
                    store(i - LAG_S)

            for j in range(NT - LAG_G, NT):
                qts[j] = gather(j)
            for j in range(NT - LAG_S, NT):
                store(j)

    nc.compile()
    return nc


def _host_consts(embed):
    bf = ml_dtypes.bfloat16
    e = np.ascontiguousarray(np.asarray(embed, dtype=np.float32))
    e_hi = e.astype(bf)
    e_lo = (e - e_hi.astype(np.float32)).astype(bf)
    ehT = np.ascontiguousarray(e_hi.T)
    elT = np.ascontiguousarray(e_lo.T)
    # -||e||^2/2 as three cascading bf16 terms (exact to ~2^-27 rel)
    t = (-0.5 * (e.astype(np.float64) ** 2).sum(1)).astype(np.float32)
    esq3 = np.zeros((3, C), dtype=bf)
    r = t
    for j in range(3):
        esq3[j] = r.astype(bf)
        r = r - esq3[j].astype(np.float32)
    return e, ehT, elT, esq3


def _in_maps(x, embed):
    e, ehT, elT, esq3 = _host_consts(embed)
    xs = np.asarray(x, dtype=np.float32).reshape(B, T, D)
    return [
        {
            "x": np.ascontiguousarray(xs[i]),
            "ehT": ehT,
            "elT": elT,
            "esq3": esq3,
            "embed": e,
        }
        for i in range(B)
    ]


def kernel(x, embed):
    global LAST_RESULTS
    from concourse import bass_utils

    if "nc" not in _CACHE:
        _CACHE["nc"] = _build()
    nc = _CACHE["nc"]

    res = bass_utils.run_bass_kernel_spmd(
        nc, _in_maps(x, embed), core_ids=list(range(B))
    )
    LAST_RESULTS = res
    outs = res.results
    q = np.stack([np.asarray(outs[i]["q"]).reshape(T, D) for i in range(B)], 0)
    ind = np.stack(
        [np.asarray(outs[i]["ind"]).reshape(T).astype(np.int32) for i in range(B)], 0
    )
    return q.astype(np.float32, copy=False), ind
